# revision 27
# baseline (speedup 1.0000x reference)
"""Trainium2 Bass kernel for nn_MultiHeadAttention (L=2048, B=4, D=1024, H=16).

Sharding: 8 cores = 4 batches x 2 head-groups (8 heads each).
Core c handles batch b=c//2, heads [512*(c%2) .. 512*(c%2)+512) of the model dim.

Per-core dataflow (S^T orientation: scores stored [k_part, q_free]):
  1. Projections (fp32r matmuls): Q^T,K^T [512e, 2048] bf16; V [2048, 512e] bf16
     with a ones-column appended per head (softmax denominator rides the ctx
     matmul as output row 64).
  2. Per head h, per q-half: scores^T tiles [128k, 1024q] in PSUM (bf16 matmul,
     K=64), exp on ScalarE with scale=1/8 folded in -> E^T bf16 SBUF.
  3. ctx' = V_aug^T . E^T accumulated over 16 k-chunks in PSUM [65, 1024];
     row 64 = softmax denominators s[q]. r = 1/s (VectorE), broadcast to
     [128, 1024] on GpSimd.
  4. ctx_norm = ctx' * Rb (VectorE), coverage^T += E^T * Rb (VectorE, bf16).
  5. out^T = Wo_slice^T . ctx_norm (bf16) -> partial [1024, 2048] per core.
Host: sums core-pair partials for out and coverage, /16 for coverage mean.
"""

import numpy as np
import sys

sys.path.insert(0, "/opt/trn_rl_repo")

import concourse.bass as bass
import concourse.mybir as mybir
import concourse.tile as tile
from concourse import bacc, bass_utils
import ml_dtypes

F32 = mybir.dt.float32
F32R = mybir.dt.float32r
BF16 = mybir.dt.bfloat16

H = 16
DH = 64
B = 4
D = 1024
HPC = 8          # heads per core
E = HPC * DH     # 512 model-dim slice per core
N_CORES = 8
P = 128

_BUILD_CACHE = {}


def build_nc(L=2048, masked=False):
    """Build the Bass program (same SPMD program for all 8 cores)."""
    KT = L // P          # k tiles of 128
    NQH = 2              # q halves
    QH = L // NQH        # q half width
    W5 = min(512, QH)    # matmul moving width (one PSUM bank of fp32)
    QC = QH // W5        # q chunks per half
    WP = min(512, L // 2)  # projection moving width (within an x stripe)
    DCH = D // P         # contraction chunks for projections

    nc = bacc.Bacc("TRN2", target_bir_lowering=False, debug=False)

    xqT_d = nc.dram_tensor("xqT", [D, L], BF16, kind="ExternalInput").ap()
    xkT_d = nc.dram_tensor("xkT", [D, L], BF16, kind="ExternalInput").ap()
    xvT_d = nc.dram_tensor("xvT", [D, L], BF16, kind="ExternalInput").ap()
    wqT_d = nc.dram_tensor("wqT", [D, E], BF16, kind="ExternalInput").ap()
    wkT_d = nc.dram_tensor("wkT", [D, E], BF16, kind="ExternalInput").ap()
    wvT_d = nc.dram_tensor("wvT", [D, E], BF16, kind="ExternalInput").ap()
    woT_d = nc.dram_tensor("woT", [E, D], BF16, kind="ExternalInput").ap()
    if masked:
        mT_d = nc.dram_tensor("maskT", [L, L], BF16, kind="ExternalInput").ap()
    outT_d = nc.dram_tensor("outT", [D, L], F32, kind="ExternalOutput").ap()
    covT_d = nc.dram_tensor("covT", [L, L], BF16, kind="ExternalOutput").ap()

    from contextlib import ExitStack

    with tile.TileContext(nc) as tc, ExitStack() as ctx:
        lp = ctx.enter_context(tc.tile_pool(name="long", bufs=1))
        qT = lp.tile([P, E // P, L], BF16, tag="qT")        # [128, 4, L]
        kT = lp.tile([P, E // P, L], BF16, tag="kT")
        vA = lp.tile([P, KT, HPC, DH + 1], BF16, tag="vA")  # V + ones col

        ones_row = lp.tile([1, P], BF16, tag="ones")
        nc.vector.memset(ones_row[:, :], 1.0)
        nc.vector.memset(vA[:, :, :, DH], 1.0)

        # ---- phase A: load weights + x, projections ----
        with tc.tile_pool(name="wz", bufs=1) as wz, \
             tc.tile_pool(name="xz", bufs=8) as xz, \
             tc.tile_pool(name="pjp", bufs=4, space="PSUM") as pjp:
            w_sb = {}
            for nm, d_ap in (("wq", wqT_d), ("wk", wkT_d), ("wv", wvT_d)):
                t = wz.tile([P, DCH, E], BF16, tag=nm)
                nc.sync.dma_start(
                    out=t[:, :, :], in_=d_ap.rearrange("(c p) e -> p c e", p=P)
                )
                w_sb[nm] = t

            XH = L // 2  # x stripe width

            def load_x(d_ap, x0):
                chs = []
                for c in range(DCH):
                    t = xz.tile([P, XH], BF16, tag="x")
                    nc.gpsimd.dma_start(
                        out=t[:, :], in_=d_ap[c * P:(c + 1) * P, x0:x0 + XH]
                    )
                    chs.append(t)
                return chs

            for xh in range(2):
                x0 = xh * XH
                # Q^T and K^T: out[e_tile, q] = sum_d W^T[d, e] X^T[d, q]
                for nm, xd, dst in (("wq", xqT_d, qT), ("wk", xkT_d, kT)):
                    xch = load_x(xd, x0)
                    for et in range(E // P):
                        for q5 in range(XH // WP):
                            ps = pjp.tile([P, WP], F32, tag="pj")
                            for c in range(DCH):
                                nc.tensor.matmul(
                                    out=ps[:, :],
                                    lhsT=w_sb[nm][:, c, et * P:(et + 1) * P],
                                    rhs=xch[c][:, q5 * WP:(q5 + 1) * WP],
                                    start=(c == 0),
                                    stop=(c == DCH - 1),
                                )
                            nc.scalar.copy(
                                out=dst[:, et, x0 + q5 * WP:x0 + (q5 + 1) * WP],
                                in_=ps[:, :],
                            )
                # V: out[l_tile, e] = sum_d X^T[d, l] W^T[d, e]
                xch = load_x(xvT_d, x0)
                for lt in range(XH // P):
                    ps = pjp.tile([P, E], F32, tag="pj")
                    for c in range(DCH):
                        nc.tensor.matmul(
                            out=ps[:, :],
                            lhsT=xch[c][:, lt * P:(lt + 1) * P],
                            rhs=w_sb["wv"][:, c, :],
                            start=(c == 0),
                            stop=(c == DCH - 1),
                        )
                    nc.scalar.copy(
                        out=vA[:, x0 // P + lt, :, 0:DH],
                        in_=ps[:, :].rearrange("p (h j) -> p h j", h=HPC),
                    )

        # ---- phase B/C pools (created after phase-A pools release SBUF) ----
        cov = lp.tile([P, KT, QH], BF16, tag="cov")
        ctxs = lp.tile([P, E // P, QH], BF16, tag="ctxs")   # ctx^T store
        woS = lp.tile([P, E // P, D], BF16, tag="woS")      # WoT chunks
        nc.sync.dma_start(
            out=woS[:, :, :], in_=woT_d.rearrange("(c p) e -> p c e", p=P)
        )
        if masked:
            mS = lp.tile([P, KT, QH], BF16, tag="mS")
            ident = lp.tile([P, P], BF16, tag="ident")
            from concourse.masks import make_identity
            make_identity(nc, ident[:, :])

        ep = ctx.enter_context(tc.tile_pool(name="eT", bufs=2))
        rp = ctx.enter_context(tc.tile_pool(name="rb", bufs=2))
        pp = ctx.enter_context(tc.tile_pool(name="pt", bufs=2))
        op = ctx.enter_context(tc.tile_pool(name="ost", bufs=2))

        with tc.tile_pool(name="mmp", bufs=2, space="PSUM") as mmp, \
             tc.tile_pool(name="ctxp", bufs=2, space="PSUM") as ctxp:
            for half in range(NQH):
                q0 = half * QH
                if masked:
                    nc.sync.dma_start(
                        out=mS[:, :, :],
                        in_=mT_d[:, q0:q0 + QH].rearrange("(t p) q -> p t q", p=P),
                    )
                for h in range(HPC):
                    m = h % 2
                    et = h // 2
                    eT = ep.tile([P, KT, QH], BF16, tag="eT")
                    cx = ctxp.tile([P, QH], F32, tag="ctx")
                    # scores^T + exp per k-tile
                    for kt in range(KT):
                        sp = mmp.tile([P, QH], F32, tag="mm")
                        for q5 in range(QC):
                            nc.tensor.matmul(
                                out=sp[:, q5 * W5:(q5 + 1) * W5],
                                lhsT=kT[64 * m:64 * m + 64, et, kt * P:(kt + 1) * P],
                                rhs=qT[64 * m:64 * m + 64, et,
                                       q0 + q5 * W5:q0 + (q5 + 1) * W5],
                                start=True,
                                stop=not masked,
                            )
                            if masked:
                                nc.tensor.matmul(
                                    out=sp[:, q5 * W5:(q5 + 1) * W5],
                                    lhsT=ident[:, :],
                                    rhs=mS[:, kt, q5 * W5:(q5 + 1) * W5],
                                    start=False,
                                    stop=True,
                                )
                        nc.scalar.activation(
                            out=eT[:, kt, :], in_=sp[:, :],
                            func=mybir.ActivationFunctionType.Exp,
                            bias=0.0, scale=0.125,
                        )
                    # ctx' accumulation (+ denominator row 64)
                    for kt in range(KT):
                        for q5 in range(QC):
                            nc.tensor.matmul(
                                out=cx[0:DH + 1, q5 * W5:(q5 + 1) * W5],
                                lhsT=vA[:, kt, h, :],
                                rhs=eT[:, kt, q5 * W5:(q5 + 1) * W5],
                                start=(kt == 0),
                                stop=(kt == KT - 1),
                            )
                    # r row (bf16) -> broadcast tile
                    r16 = rp.tile([1, QH], BF16, tag="r16")
                    rb = rp.tile([P, QH], BF16, tag="rb")
                    with nc.allow_low_precision("bf16 r feeds bf16 consumers"):
                        nc.vector.reciprocal(out=r16[:, :], in_=cx[DH:DH + 1, :])
                    rbp = mmp.tile([P, QH], F32, tag="mm")
                    for q5 in range(QC):
                        nc.tensor.matmul(
                            out=rbp[:, q5 * W5:(q5 + 1) * W5],
                            lhsT=ones_row[:, :],
                            rhs=r16[:, q5 * W5:(q5 + 1) * W5],
                            start=True, stop=True,
                        )
                    nc.scalar.copy(out=rb[:, :], in_=rbp[:, :])
                    # normalized ctx into store
                    nc.vector.tensor_tensor(
                        out=ctxs[64 * m:64 * m + 64, et, :],
                        in0=cx[0:DH, :],
                        in1=rb[0:DH, :],
                        op=mybir.AluOpType.mult,
                    )
                    # coverage accumulation
                    for kt in range(KT):
                        if h == 0:
                            nc.vector.tensor_tensor(
                                out=cov[:, kt, :], in0=eT[:, kt, :], in1=rb[:, :],
                                op=mybir.AluOpType.mult,
                            )
                        else:
                            pt = pp.tile([P, QH], BF16, tag="pt")
                            nc.vector.tensor_tensor(
                                out=pt[:, :], in0=eT[:, kt, :], in1=rb[:, :],
                                op=mybir.AluOpType.mult,
                            )
                            nc.vector.tensor_tensor(
                                out=cov[:, kt, :], in0=cov[:, kt, :], in1=pt[:, :],
                                op=mybir.AluOpType.add,
                            )
                # coverage out
                for kt in range(KT):
                    nc.sync.dma_start(
                        out=covT_d[kt * P:(kt + 1) * P, q0:q0 + QH], in_=cov[:, kt, :]
                    )
                # out-projection for this half
                for eo in range(D // P):
                    ps = mmp.tile([P, QH], F32, tag="mm")
                    for q5 in range(QC):
                        for c in range(E // P):
                            nc.tensor.matmul(
                                out=ps[:, q5 * W5:(q5 + 1) * W5],
                                lhsT=woS[:, c, eo * P:(eo + 1) * P],
                                rhs=ctxs[:, c, q5 * W5:(q5 + 1) * W5],
                                start=(c == 0),
                                stop=(c == E // P - 1),
                            )
                    ot = op.tile([P, QH], F32, tag="ot")
                    nc.scalar.copy(out=ot[:, :], in_=ps[:, :])
                    nc.sync.dma_start(
                        out=outT_d[eo * P:(eo + 1) * P, q0:q0 + QH], in_=ot[:, :]
                    )
    nc.compile()
    return nc


def _get_nc(L, masked):
    key = (L, masked)
    if key not in _BUILD_CACHE:
        _BUILD_CACHE[key] = build_nc(L, masked)
    return _BUILD_CACHE[key]


def make_in_maps(query, key, value, Wq, Wk, Wv, Wo, mask=None):
    """Per-core input dicts. Core c: batch c//2, head-group c%2."""
    in_maps = []
    bf16 = ml_dtypes.bfloat16
    for c in range(N_CORES):
        b, g = c // 2, c % 2
        e0 = E * g
        m = {
            "xqT": np.ascontiguousarray(query[:, b, :].T).astype(bf16),
            "xkT": np.ascontiguousarray(key[:, b, :].T).astype(bf16),
            "xvT": np.ascontiguousarray(value[:, b, :].T).astype(bf16),
            "wqT": np.ascontiguousarray(Wq[e0:e0 + E, :].T).astype(bf16),
            "wkT": np.ascontiguousarray(Wk[e0:e0 + E, :].T).astype(bf16),
            "wvT": np.ascontiguousarray(Wv[e0:e0 + E, :].T).astype(bf16),
            "woT": np.ascontiguousarray(Wo[:, e0:e0 + E].T).astype(bf16),
        }
        if mask is not None:
            mt = np.where(mask[b].T, np.float32(-240000.0), np.float32(0.0))
            m["maskT"] = mt.astype(ml_dtypes.bfloat16)
        in_maps.append(m)
    return in_maps


def gather_outputs(results, L):
    out = np.empty((L, B, D), dtype=np.float32)
    coverage = np.empty((B, L, L), dtype=np.float32)
    for b in range(B):
        r0, r1 = results[2 * b], results[2 * b + 1]
        out[:, b, :] = (r0["outT"] + r1["outT"]).T
        cov = (r0["covT"].astype(np.float32) + r1["covT"].astype(np.float32)) / 16.0
        coverage[b] = cov.T
    return out, coverage


def kernel(query, key, value, mask, Wq, Wk, Wv, Wo):
    query = np.asarray(query, dtype=np.float32)
    key = np.asarray(key, dtype=np.float32)
    value = np.asarray(value, dtype=np.float32)
    mask = np.asarray(mask)
    L = query.shape[0]
    masked = bool(mask.any())
    nc = _get_nc(L, masked)
    in_maps = make_in_maps(
        query, key, value, np.asarray(Wq, np.float32), np.asarray(Wk, np.float32),
        np.asarray(Wv, np.float32), np.asarray(Wo, np.float32),
        mask if masked else None,
    )
    res = bass_utils.run_bass_kernel_spmd(nc, in_maps, core_ids=list(range(N_CORES)))
    return gather_outputs(res.results, L)


# revision 72
# speedup vs baseline: 1.0866x; 1.0866x over previous
"""Trainium2 Bass kernel for nn_MultiHeadAttention (L=2048, B=4, D=1024, H=16).

Sharding: 8 cores = 4 batches x 2 head-groups (8 heads each).
Core c handles batch b=c//2, heads [512*(c%2) .. 512*(c%2)+512) of the model dim.

Per-core dataflow (S^T orientation: scores stored [k_part, q_free]):
  1. Projections (fp32r matmuls): Q^T,K^T [512e, 2048] bf16; V [2048, 512e] bf16
     with a ones-column appended per head (softmax denominator rides the ctx
     matmul as output row 64).
  2. Per head h, per q-half: scores^T tiles [128k, 1024q] in PSUM (bf16 matmul,
     K=64), exp on ScalarE with scale=1/8 folded in -> E^T bf16 SBUF.
  3. ctx' = V_aug^T . E^T accumulated over 16 k-chunks in PSUM [65, 1024];
     row 64 = softmax denominators s[q]. r = 1/s (VectorE), broadcast to
     [128, 1024] on GpSimd.
  4. ctx_norm = ctx' * Rb (VectorE), coverage^T += E^T * Rb (VectorE, bf16).
  5. out^T = Wo_slice^T . ctx_norm (bf16) -> partial [1024, 2048] per core.
Host: sums core-pair partials for out and coverage, /16 for coverage mean.
"""

import numpy as np
import sys

sys.path.insert(0, "/opt/trn_rl_repo")

import concourse.bass as bass
import concourse.mybir as mybir
import concourse.tile as tile
from concourse import bacc, bass_utils
import ml_dtypes

F32 = mybir.dt.float32
F32R = mybir.dt.float32r
BF16 = mybir.dt.bfloat16

H = 16
DH = 64
B = 4
D = 1024
HPC = 8          # heads per core
E = HPC * DH     # 512 model-dim slice per core
N_CORES = 8
P = 128

_BUILD_CACHE = {}


def build_nc(L=2048, masked=False, rb_mode="pe", mm_bufs=3, ctx_bufs=1, kq_div=4,
             et_bufs=2, cov_mode="dve"):
    """Build the Bass program (same SPMD program for all 8 cores)."""
    KT = L // P          # k tiles of 128
    NQH = 2              # q halves
    QH = L // NQH        # q half width
    W5 = min(512, QH)    # matmul moving width (one PSUM bank of fp32)
    QC = QH // W5        # q chunks per half
    WP = min(512, L // 2)  # projection moving width (within an x stripe)
    DCH = D // P         # contraction chunks for projections

    nc = bacc.Bacc("TRN2", target_bir_lowering=False, debug=False)

    xqT_d = nc.dram_tensor("xqT", [D, L], BF16, kind="ExternalInput").ap()
    xkT_d = nc.dram_tensor("xkT", [D, L], BF16, kind="ExternalInput").ap()
    xvT_d = nc.dram_tensor("xvT", [D, L], BF16, kind="ExternalInput").ap()
    wqT_d = nc.dram_tensor("wqT", [D, E], BF16, kind="ExternalInput").ap()
    wkT_d = nc.dram_tensor("wkT", [D, E], BF16, kind="ExternalInput").ap()
    wvT_d = nc.dram_tensor("wvT", [D, E], BF16, kind="ExternalInput").ap()
    woT_d = nc.dram_tensor("woT", [E, D], BF16, kind="ExternalInput").ap()
    if masked:
        mT_d = nc.dram_tensor("maskT", [L, L], BF16, kind="ExternalInput").ap()
    outT_d = nc.dram_tensor("outT", [D, L], F32, kind="ExternalOutput").ap()
    covT_d = nc.dram_tensor("covT", [L, L], BF16, kind="ExternalOutput").ap()
    rsc_d = nc.dram_tensor("rscratch", [NQH * HPC, L // NQH], BF16,
                           kind="Internal").ap()

    from contextlib import ExitStack

    with tile.TileContext(nc) as tc, ExitStack() as ctx:
        lp = ctx.enter_context(tc.tile_pool(name="long", bufs=1))
        qT = lp.tile([P, E // P, L], BF16, tag="qT")        # [128, 4, L]
        kT = lp.tile([P, E // P, L], BF16, tag="kT")
        vA = lp.tile([P, KT, HPC, DH + 1], BF16, tag="vA")  # V + ones col

        ones_row = lp.tile([1, P], BF16, tag="ones")
        nc.vector.memset(ones_row[:, :], 1.0)
        nc.vector.memset(vA[:, :, :, DH], 1.0)

        # ---- phase A: load weights + x, projections ----
        with tc.tile_pool(name="wz", bufs=1) as wz, \
             tc.tile_pool(name="xz", bufs=8) as xz, \
             tc.tile_pool(name="pjp", bufs=4, space="PSUM") as pjp:
            w_sb = {}
            for nm, d_ap in (("wq", wqT_d), ("wk", wkT_d)):
                t = wz.tile([P, DCH, E], BF16, tag=nm)
                nc.sync.dma_start(
                    out=t[:, :, :], in_=d_ap.rearrange("(c p) e -> p c e", p=P)
                )
                w_sb[nm] = t

            XH = L // 2  # x stripe width

            def load_x(d_ap, x0):
                chs = []
                for c in range(DCH):
                    t = xz.tile([P, XH], BF16, tag="x")
                    nc.sync.dma_start(
                        out=t[:, :], in_=d_ap[c * P:(c + 1) * P, x0:x0 + XH]
                    )
                    chs.append(t)
                return chs

            for xh in range(2):
                x0 = xh * XH
                # Q^T and K^T: out[e_tile, q] = sum_d W^T[d, e] X^T[d, q]
                for nm, xd, dst in (("wk", xkT_d, kT), ("wq", xqT_d, qT)):
                    xch = load_x(xd, x0)
                    for et in range(E // P):
                        for q5 in range(XH // WP):
                            ps = pjp.tile([P, WP], F32, tag="pj")
                            for c in range(DCH):
                                nc.tensor.matmul(
                                    out=ps[:, :],
                                    lhsT=w_sb[nm][:, c, et * P:(et + 1) * P],
                                    rhs=xch[c][:, q5 * WP:(q5 + 1) * WP],
                                    start=(c == 0),
                                    stop=(c == DCH - 1),
                                )
                            dst_ap = dst[:, et, x0 + q5 * WP:x0 + (q5 + 1) * WP]
                            if (et + q5) % 2 == 0:
                                nc.vector.tensor_copy(out=dst_ap, in_=ps[:, :])
                            else:
                                nc.scalar.copy(out=dst_ap, in_=ps[:, :])
                if masked:
                    # masked variant: mS occupies the deferred-V SBUF window,
                    # so project V up front as part of phase A
                    wv = wz.tile([P, DCH, E], BF16, tag="wv")
                    nc.sync.dma_start(
                        out=wv[:, :, :],
                        in_=wvT_d.rearrange("(c p) e -> p c e", p=P),
                    )
                    xch = load_x(xvT_d, x0)
                    for lt in range(XH // P):
                        ps = pjp.tile([P, E], F32, tag="pj")
                        for c in range(DCH):
                            nc.tensor.matmul(
                                out=ps[:, :],
                                lhsT=xch[c][:, lt * P:(lt + 1) * P],
                                rhs=wv[:, c, :],
                                start=(c == 0),
                                stop=(c == DCH - 1),
                            )
                        nc.vector.tensor_copy(
                            out=vA[:, x0 // P + lt, :, 0:DH],
                            in_=ps[:, :].rearrange("p (hh j) -> p hh j", hh=HPC),
                        )

        # ---- phase B/C pools (created after phase-A pools release SBUF) ----
        if masked:
            # allocate everything up front (V was projected in phase A)
            cov = lp.tile([P, KT, QH], BF16, tag="cov")
            ctxs = lp.tile([P, E // P, QH], BF16, tag="ctxs")
            woS = lp.tile([P, E // P, D], BF16, tag="woS")
            nc.sync.dma_start(
                out=woS[:, :, :], in_=woT_d.rearrange("(c p) e -> p c e", p=P)
            )
            mS = lp.tile([P, KT, QH], BF16, tag="mS")
            ident = lp.tile([P, P], BF16, tag="ident")
            from concourse.masks import make_identity
            make_identity(nc, ident[:, :])
        else:
            cov = ctxs = woS = None  # allocated after the deferred V window

        ep = ctx.enter_context(tc.tile_pool(name="eT", bufs=et_bufs))
        _pools = {}

        def get_pool(name, bufs, space="SBUF"):
            if name not in _pools:
                _pools[name] = ctx.enter_context(
                    tc.tile_pool(name=name, bufs=bufs, space=space))
            return _pools[name]

        mmp = ctx.enter_context(
            tc.tile_pool(name="mmp", bufs=mm_bufs, space="PSUM"))
        if True:
            for half in range(NQH):
                q0 = half * QH
                if masked:
                    nc.sync.dma_start(
                        out=mS[:, :, :],
                        in_=mT_d[:, q0:q0 + QH].rearrange("(t p) q -> p t q", p=P),
                    )
                for h in range(HPC):
                    m = h % 2
                    et = h // 2
                    eT = ep.tile([P, KT, QH], BF16, tag="eT")
                    # scores^T + exp per k-tile
                    for kt in range(KT):
                        sp = mmp.tile([P, QH], F32, tag="mm")
                        for q5 in range(QC):
                            nc.tensor.matmul(
                                out=sp[:, q5 * W5:(q5 + 1) * W5],
                                lhsT=kT[64 * m:64 * m + 64, et, kt * P:(kt + 1) * P],
                                rhs=qT[64 * m:64 * m + 64, et,
                                       q0 + q5 * W5:q0 + (q5 + 1) * W5],
                                start=True,
                                stop=not masked,
                            )
                            if masked:
                                nc.tensor.matmul(
                                    out=sp[:, q5 * W5:(q5 + 1) * W5],
                                    lhsT=ident[:, :],
                                    rhs=mS[:, kt, q5 * W5:(q5 + 1) * W5],
                                    start=False,
                                    stop=True,
                                )
                        nc.scalar.activation(
                            out=eT[:, kt, :], in_=sp[:, :],
                            func=mybir.ActivationFunctionType.Exp,
                            bias=0.0, scale=0.125,
                        )
                    if not masked and half == 0 and h == 0:
                        # Deferred V projection: PE projects V while ScalarE
                        # runs head 0's exp. out[l_tile, e] = sum_d XvT WvT.
                        with tc.tile_pool(name="wvz", bufs=1) as wvz, \
                             tc.tile_pool(name="xz2", bufs=8) as xz2, \
                             tc.tile_pool(name="vps", bufs=2,
                                          space="PSUM") as vps:
                            wv = wvz.tile([P, DCH, E], BF16, tag="wv")
                            nc.sync.dma_start(
                                out=wv[:, :, :],
                                in_=wvT_d.rearrange("(c p) e -> p c e", p=P),
                            )
                            XH2 = L // 2
                            for xh in range(2):
                                x0 = xh * XH2
                                xch = []
                                for c in range(DCH):
                                    t = xz2.tile([P, XH2], BF16, tag="x2")
                                    nc.sync.dma_start(
                                        out=t[:, :],
                                        in_=xvT_d[c * P:(c + 1) * P, x0:x0 + XH2],
                                    )
                                    xch.append(t)
                                for lt in range(XH2 // P):
                                    ps = vps.tile([P, E], F32, tag="vp")
                                    for c in range(DCH):
                                        nc.tensor.matmul(
                                            out=ps[:, :],
                                            lhsT=xch[c][:, lt * P:(lt + 1) * P],
                                            rhs=wv[:, c, :],
                                            start=(c == 0),
                                            stop=(c == DCH - 1),
                                        )
                                    nc.vector.tensor_copy(
                                        out=vA[:, x0 // P + lt, :, 0:DH],
                                        in_=ps[:, :].rearrange(
                                            "p (hh j) -> p hh j", hh=HPC),
                                    )
                        # V window closed: now safe to allocate the big
                        # phase-B tensors
                        cov = lp.tile([P, KT, QH], BF16, tag="cov")
                        ctxs = lp.tile([P, E // P, QH], BF16, tag="ctxs")
                        woS = lp.tile([P, E // P, D], BF16, tag="woS")
                        nc.sync.dma_start(
                            out=woS[:, :, :],
                            in_=woT_d.rearrange("(c p) e -> p c e", p=P),
                        )
                    # ctx' accumulation (+ denominator row 64)
                    cx = get_pool("ctxp", ctx_bufs, "PSUM").tile(
                        [P, QH], F32, tag="ctx")
                    for kt in range(KT):
                        for q5 in range(QC):
                            nc.tensor.matmul(
                                out=cx[0:DH + 1, q5 * W5:(q5 + 1) * W5],
                                lhsT=vA[:, kt, h, :],
                                rhs=eT[:, kt, q5 * W5:(q5 + 1) * W5],
                                start=(kt == 0),
                                stop=(kt == KT - 1),
                            )
                    # r row (bf16), broadcast via PE outer product with ones
                    r16 = get_pool("rb", 2).tile([1, QH], BF16, tag="r16")
                    rb = get_pool("rb", 2).tile([P, QH], BF16, tag="rb")
                    with nc.allow_low_precision("bf16 r feeds bf16 consumers"):
                        nc.vector.reciprocal(out=r16[:, :], in_=cx[DH:DH + 1, :])
                    if rb_mode == "pe":
                        rbp = mmp.tile([P, QH], F32, tag="mm")
                        for q5 in range(QC):
                            nc.tensor.matmul(
                                out=rbp[:, q5 * W5:(q5 + 1) * W5],
                                lhsT=ones_row[:, :],
                                rhs=r16[:, q5 * W5:(q5 + 1) * W5],
                                start=True, stop=True,
                            )
                        nc.scalar.copy(out=rb[:, :], in_=rbp[:, :])
                    else:
                        ridx = half * HPC + h
                        nc.sync.dma_start(out=rsc_d[ridx:ridx + 1, :], in_=r16[:, :])
                        nc.sync.dma_start(
                            out=rb[:, :],
                            in_=rsc_d[ridx:ridx + 1, :].to_broadcast((P, QH)),
                        )
                    # normalized ctx into store
                    nc.vector.tensor_tensor(
                        out=ctxs[64 * m:64 * m + 64, et, :],
                        in0=cx[0:DH, :],
                        in1=rb[0:DH, :],
                        op=mybir.AluOpType.mult,
                    )
                    # coverage accumulation: per-head normalize (DVE mul) and
                    # head-sum. cov_mode "dma" rides CCE-accumulate DMAs;
                    # "dve" (hardware-proven) adds on VectorE.
                    kq = max(1, KT // kq_div)
                    pt = None
                    for kt in range(KT):
                        if h == 0:
                            nc.vector.tensor_tensor(
                                out=cov[:, kt, :], in0=eT[:, kt, :], in1=rb[:, :],
                                op=mybir.AluOpType.mult,
                            )
                            continue
                        if cov_mode == "dma":
                            if kt % kq == 0:
                                pt = get_pool("pt", 2).tile(
                                    [P, kq, QH], BF16, tag="pt")
                            nc.vector.tensor_tensor(
                                out=pt[:, kt % kq, :], in0=eT[:, kt, :],
                                in1=rb[:, :], op=mybir.AluOpType.mult,
                            )
                            if kt % kq == kq - 1:
                                k0 = kt - kq + 1
                                nc.gpsimd.dma_start(
                                    out=cov[:, k0:kt + 1, :], in_=pt[:, :, :],
                                    accum_op=mybir.AluOpType.add,
                                )
                        else:
                            nc.vector.tensor_tensor(
                                out=eT[:, kt, :], in0=eT[:, kt, :], in1=rb[:, :],
                                op=mybir.AluOpType.mult,
                            )
                            nc.vector.tensor_tensor(
                                out=cov[:, kt, :], in0=cov[:, kt, :],
                                in1=eT[:, kt, :], op=mybir.AluOpType.add,
                            )
                # coverage out
                for kt in range(KT):
                    nc.sync.dma_start(
                        out=covT_d[kt * P:(kt + 1) * P, q0:q0 + QH], in_=cov[:, kt, :]
                    )
                # out-projection for this half
                for eo in range(D // P):
                    ps = mmp.tile([P, QH], F32, tag="mm")
                    for q5 in range(QC):
                        for c in range(E // P):
                            nc.tensor.matmul(
                                out=ps[:, q5 * W5:(q5 + 1) * W5],
                                lhsT=woS[:, c, eo * P:(eo + 1) * P],
                                rhs=ctxs[:, c, q5 * W5:(q5 + 1) * W5],
                                start=(c == 0),
                                stop=(c == E // P - 1),
                            )
                    ot = get_pool("ost", 1 if masked else 2).tile(
                        [P, QH], F32, tag="ot")
                    nc.scalar.copy(out=ot[:, :], in_=ps[:, :])
                    nc.sync.dma_start(
                        out=outT_d[eo * P:(eo + 1) * P, q0:q0 + QH], in_=ot[:, :]
                    )
    nc.compile()
    return nc


def _get_nc(L, masked):
    key = (L, masked)
    if key not in _BUILD_CACHE:
        # masked keeps mS resident, so shrink the pt staging buffer
        _BUILD_CACHE[key] = build_nc(L, masked, kq_div=16 if masked else 4)
    return _BUILD_CACHE[key]


def make_in_maps(query, key, value, Wq, Wk, Wv, Wo, mask=None):
    """Per-core input dicts. Core c: batch c//2, head-group c%2."""
    in_maps = []
    bf16 = ml_dtypes.bfloat16
    for c in range(N_CORES):
        b, g = c // 2, c % 2
        e0 = E * g
        m = {
            "xqT": np.ascontiguousarray(query[:, b, :].T).astype(bf16),
            "xkT": np.ascontiguousarray(key[:, b, :].T).astype(bf16),
            "xvT": np.ascontiguousarray(value[:, b, :].T).astype(bf16),
            "wqT": np.ascontiguousarray(Wq[e0:e0 + E, :].T).astype(bf16),
            "wkT": np.ascontiguousarray(Wk[e0:e0 + E, :].T).astype(bf16),
            "wvT": np.ascontiguousarray(Wv[e0:e0 + E, :].T).astype(bf16),
            "woT": np.ascontiguousarray(Wo[:, e0:e0 + E].T).astype(bf16),
        }
        if mask is not None:
            mt = np.where(mask[b].T, np.float32(-240000.0), np.float32(0.0))
            m["maskT"] = mt.astype(ml_dtypes.bfloat16)
        in_maps.append(m)
    return in_maps


def gather_outputs(results, L):
    out = np.empty((L, B, D), dtype=np.float32)
    coverage = np.empty((B, L, L), dtype=np.float32)
    for b in range(B):
        r0, r1 = results[2 * b], results[2 * b + 1]
        out[:, b, :] = (r0["outT"] + r1["outT"]).T
        cov = (r0["covT"].astype(np.float32) + r1["covT"].astype(np.float32)) / 16.0
        coverage[b] = cov.T
    return out, coverage


def kernel(query, key, value, mask, Wq, Wk, Wv, Wo):
    query = np.asarray(query, dtype=np.float32)
    key = np.asarray(key, dtype=np.float32)
    value = np.asarray(value, dtype=np.float32)
    mask = np.asarray(mask)
    L = query.shape[0]
    masked = bool(mask.any())
    nc = _get_nc(L, masked)
    in_maps = make_in_maps(
        query, key, value, np.asarray(Wq, np.float32), np.asarray(Wk, np.float32),
        np.asarray(Wv, np.float32), np.asarray(Wo, np.float32),
        mask if masked else None,
    )
    res = bass_utils.run_bass_kernel_spmd(nc, in_maps, core_ids=list(range(N_CORES)))
    return gather_outputs(res.results, L)
